# revision 7
# baseline (speedup 1.0000x reference)
"""NetVLAD forward on 8 Trainium2 NeuronCores (Bass/Tile).

Data-parallel over batch: B=32 -> 4 batches per core. Per batch, stream
N=8192 tokens in 128-token tiles:
  1. sumsq + rsqrt -> per-token rnorm           (DVE/ACT)
  2. xn = x * rnorm  (bf16, fused f32->bf16)    (GPSIMD)
  3. xnT = PE-transpose(xn) -> PSUM -> SBUF     (PE + DVE copy)
  4. logits = xnT.T @ wT + conv_b  (bias pre-filled in PSUM by a rank-1
     matmul, mm1 accumulates on top)            (PE)
  5. e = exp(logits), s = row-sum via ACT accum (ACT)
  6. a = e * (1/s)  (bf16)                      (DVE)
  7. ax[k, 0:128] += a.T @ [xn | ones] accumulated in PSUM over all 64
     tiles of the batch; column 128 gives a_sum (PE)
Finalize per batch: vlad = ax - centroids*a_sum, intra-L2-normalize rows,
scale by 1/sqrt(K)=0.125 for the global L2 norm (rows are unit after
intra-norm, so the global norm is sqrt(K) exactly up to fp rounding).
"""

import functools
from contextlib import ExitStack

import numpy as np

import concourse.bass as bass
import concourse.tile as tile
from concourse import bacc, masks, mybir
from concourse.bass_utils import run_bass_kernel_spmd

B, N, D, K = 32, 8192, 128, 64
NCORES = 8
BPC = B // NCORES            # 4 batches per core
P = 128                      # token tile size = partitions
NT_CHUNK = 16                # token tiles per DMA chunk (2048 tokens, 1 MiB)
NT_GROUP = 8                 # token tiles per softmax/psum group
TILES = N // P               # 64 token tiles per batch
CHUNKS = TILES // NT_CHUNK   # 4
GROUPS = NT_CHUNK // NT_GROUP  # 2 groups per chunk

F32 = mybir.dt.float32
BF16 = mybir.dt.bfloat16
EPS = 1e-12
MULT = mybir.AluOpType.mult
ADD = mybir.AluOpType.add


def _build_kernel(bpc=BPC, n=N, num_devices=NCORES, repeat=1):
    tiles = n // P
    chunks = tiles // NT_CHUNK
    assert chunks * NT_CHUNK == tiles
    nc = bacc.Bacc(
        "TRN2", target_bir_lowering=False, debug=False, num_devices=num_devices
    )
    x_d = nc.dram_tensor("x", [bpc, n, D], F32, kind="ExternalInput").ap()
    cent_d = nc.dram_tensor("centroids", [K, D], F32, kind="ExternalInput").ap()
    cw_d = nc.dram_tensor("conv_w", [K, D], F32, kind="ExternalInput").ap()
    cb_d = nc.dram_tensor("conv_b", [1, K], F32, kind="ExternalInput").ap()
    y_d = nc.dram_tensor("y", [bpc, K, D], F32, kind="ExternalOutput").ap()

    with tile.TileContext(nc) as tc, ExitStack() as ctx:
        const = ctx.enter_context(tc.tile_pool(name="const", bufs=1))
        ident_bf = const.tile([P, P], BF16)
        masks.make_identity(nc, ident_bf[:])
        ident_f32 = const.tile([P, P], F32)
        masks.make_identity(nc, ident_f32[:])
        ones_row = const.tile([1, P], BF16)
        nc.gpsimd.memset(ones_row[:], 1.0)

        cent_sb = const.tile([K, D], F32)
        nc.sync.dma_start(cent_sb[:], cent_d)
        cw_sb = const.tile([K, D], F32)
        nc.sync.dma_start(cw_sb[:], cw_d)
        cb_f32 = const.tile([1, K], F32)
        nc.sync.dma_start(cb_f32[:], cb_d)
        cb_bf = const.tile([1, K], BF16)
        nc.vector.tensor_copy(cb_bf[:], cb_f32[:])

        # wT = conv_w.T  [D, K] in bf16 (one-time PE transpose)
        ps_init = ctx.enter_context(
            tc.tile_pool(name="ps_init", bufs=1, space="PSUM")
        )
        cwT_ps = ps_init.tile([D, K], F32)
        nc.tensor.transpose(cwT_ps[:], cw_sb[:], ident_f32[0:K, 0:K])
        wT_bf = const.tile([D, K], BF16)
        nc.vector.tensor_copy(wT_bf[:], cwT_ps[:])

        x_pool = ctx.enter_context(tc.tile_pool(name="x", bufs=2))
        xn_pool = ctx.enter_context(tc.tile_pool(name="xn", bufs=2))
        sq_pool = ctx.enter_context(tc.tile_pool(name="sq", bufs=2))
        stat_pool = ctx.enter_context(tc.tile_pool(name="stat", bufs=3))
        e_pool = ctx.enter_context(tc.tile_pool(name="e", bufs=NT_GROUP + 2))
        ep_pool = ctx.enter_context(tc.tile_pool(name="ep", bufs=3))
        xts_pool = ctx.enter_context(tc.tile_pool(name="xts", bufs=3))
        fin_pool = ctx.enter_context(tc.tile_pool(name="fin", bufs=2))

        xt_psum = ctx.enter_context(tc.tile_pool(name="xt_ps", bufs=2, space="PSUM"))
        lg_psum = ctx.enter_context(tc.tile_pool(name="lg_ps", bufs=2, space="PSUM"))
        ax_psum = ctx.enter_context(tc.tile_pool(name="ax_ps", bufs=2, space="PSUM"))

        rep_ctx = tc.For_i(0, repeat, 1) if repeat > 1 else None
        if rep_ctx is not None:
            rep_ctx.__enter__()

        for b in range(bpc):
            ax_ps = ax_psum.tile([K, D + 1], F32, tag="ax")
            jj = 0
            for c in range(chunks):
                xs = x_pool.tile([P, NT_CHUNK, D], F32, tag="xs")
                src = x_d[b, c * NT_CHUNK * P:(c + 1) * NT_CHUNK * P, :]
                nc.sync.dma_start(xs[:], src.rearrange("(nt p) d -> p nt d", p=P))
                xn = xn_pool.tile([P, NT_CHUNK, D + 1], BF16, tag="xn")
                nc.gpsimd.memset(xn[:, :, D:D + 1], 1.0)

                for g in range(GROUPS):
                    ss = stat_pool.tile([P, NT_GROUP], F32, tag="ss")
                    for j in range(NT_GROUP):
                        t = g * NT_GROUP + j
                        sq = sq_pool.tile([P, D], F32, tag="sq")
                        nc.vector.scalar_tensor_tensor(
                            out=sq[:], in0=xs[:, t, :], scalar=1.0,
                            in1=xs[:, t, :], op0=MULT, op1=MULT,
                            accum_out=ss[:, j:j + 1],
                        )
                    nrm = stat_pool.tile([P, NT_GROUP], F32, tag="nrm")
                    nc.scalar.activation(
                        nrm[:], ss[:], mybir.ActivationFunctionType.Sqrt
                    )
                    rnorm = stat_pool.tile([P, NT_GROUP], F32, tag="rnorm")
                    nc.vector.tensor_scalar_max(nrm[:], nrm[:], EPS)
                    nc.vector.reciprocal(rnorm[:], nrm[:])

                    lg = lg_psum.tile([P, NT_GROUP * K], F32, tag="lg")
                    s8 = stat_pool.tile([P, NT_GROUP], F32, tag="s8")
                    e_tiles = []
                    for j in range(NT_GROUP):
                        t = g * NT_GROUP + j
                        nc.gpsimd.tensor_scalar(
                            out=xn[:, t, 0:D], in0=xs[:, t, :],
                            scalar1=rnorm[:, j:j + 1], scalar2=None, op0=MULT,
                        )
                        xt_ps = xt_psum.tile([P, P], BF16, tag="xtp")
                        nc.tensor.transpose(xt_ps[:], xn[:, t, 0:D], ident_bf[:])
                        xt_sb = xts_pool.tile([P, P], BF16, tag="xts")
                        nc.vector.tensor_copy(xt_sb[:], xt_ps[:])
                        nc.tensor.matmul(
                            lg[:, j * K:(j + 1) * K], lhsT=ones_row[:],
                            rhs=cb_bf[:], start=True, stop=False,
                        )
                        nc.tensor.matmul(
                            lg[:, j * K:(j + 1) * K], lhsT=xt_sb[:],
                            rhs=wT_bf[:], start=False, stop=True,
                        )
                        e_sb = e_pool.tile([P, K], BF16, tag="e")
                        nc.scalar.activation(
                            e_sb[:], lg[:, j * K:(j + 1) * K],
                            mybir.ActivationFunctionType.Exp,
                            accum_out=s8[:, j:j + 1],
                        )
                        e_tiles.append(e_sb)
                    r8 = stat_pool.tile([P, NT_GROUP], F32, tag="r8")
                    nc.vector.reciprocal(r8[:], s8[:])
                    for j in range(NT_GROUP):
                        t = g * NT_GROUP + j
                        ep = ep_pool.tile([P, K], BF16, tag="ep")
                        nc.vector.tensor_scalar_mul(
                            ep[:], e_tiles[j][:], r8[:, j:j + 1]
                        )
                        nc.tensor.matmul(
                            ax_ps[:], lhsT=ep[:], rhs=xn[:, t, :],
                            start=(jj == 0), stop=(jj == tiles - 1),
                        )
                        jj += 1

            # ---- finalize batch b ----
            asum_sb = fin_pool.tile([K, 1], F32, tag="asum")
            nc.vector.tensor_copy(asum_sb[:], ax_ps[:, D:D + 1])
            tmp = fin_pool.tile([K, D], F32, tag="tmp")
            nc.vector.tensor_scalar_mul(tmp[:], cent_sb[:], asum_sb[:])
            vlad = fin_pool.tile([K, D], F32, tag="vlad")
            nc.vector.tensor_sub(vlad[:], ax_ps[:, 0:D], tmp[:])
            sqv = fin_pool.tile([K, D], F32, tag="sqv")
            rss = fin_pool.tile([K, 1], F32, tag="rss")
            nc.vector.scalar_tensor_tensor(
                out=sqv[:], in0=vlad[:], scalar=1.0, in1=vlad[:],
                op0=MULT, op1=MULT, accum_out=rss[:],
            )
            nrm2 = fin_pool.tile([K, 1], F32, tag="nrm2")
            nc.scalar.activation(
                nrm2[:], rss[:], mybir.ActivationFunctionType.Sqrt
            )
            nc.vector.tensor_scalar_max(nrm2[:], nrm2[:], EPS)
            rn2 = fin_pool.tile([K, 1], F32, tag="rn2")
            nc.vector.reciprocal(rn2[:], nrm2[:])
            yb = fin_pool.tile([K, D], F32, tag="yb")
            nc.vector.tensor_scalar(
                out=yb[:], in0=vlad[:], scalar1=rn2[:], scalar2=0.125,
                op0=MULT, op1=MULT,
            )
            nc.sync.dma_start(y_d[b], yb[:])

        if rep_ctx is not None:
            rep_ctx.__exit__(None, None, None)

    nc.compile()
    return nc


@functools.cache
def _get_kernel():
    return _build_kernel()


def kernel(x, centroids, conv_w, conv_b, **kw):
    x = np.ascontiguousarray(np.asarray(x, dtype=np.float32))
    centroids = np.ascontiguousarray(np.asarray(centroids, dtype=np.float32))
    conv_w = np.ascontiguousarray(np.asarray(conv_w, dtype=np.float32))
    conv_b = np.ascontiguousarray(
        np.asarray(conv_b, dtype=np.float32).reshape(1, K)
    )
    nc = _get_kernel()
    in_maps = [
        {
            "x": x[i * BPC:(i + 1) * BPC],
            "centroids": centroids,
            "conv_w": conv_w,
            "conv_b": conv_b,
        }
        for i in range(NCORES)
    ]
    res = run_bass_kernel_spmd(nc, in_maps, core_ids=list(range(NCORES)))
    y = np.concatenate([res.results[i]["y"] for i in range(NCORES)], axis=0)
    return y.reshape(B, K * D)


if __name__ == "__main__":
    rng = np.random.default_rng(0)
    out = kernel(
        x=rng.standard_normal((B, N, D), dtype=np.float32),
        centroids=rng.standard_normal((K, D), dtype=np.float32) * 0.01,
        conv_w=rng.standard_normal((K, D), dtype=np.float32) / np.sqrt(D),
        conv_b=rng.standard_normal((K,), dtype=np.float32) * 0.01,
    )
    print(out.shape, out.dtype, float(np.abs(out).max()))


# revision 13
# speedup vs baseline: 1.0625x; 1.0625x over previous
"""NetVLAD forward on 8 Trainium2 NeuronCores (Bass/Tile).

Data-parallel over batch: B=32 -> 4 batches per core. Per batch, stream
N=8192 tokens in 128-token tiles:
  1. sumsq + rsqrt -> per-token rnorm           (DVE/ACT)
  2. xn = x * rnorm  (bf16, fused f32->bf16)    (GPSIMD)
  3. xnT = PE-transpose(xn) -> PSUM -> SBUF     (PE + DVE copy)
  4. logits = xnT.T @ wT + conv_b  (bias pre-filled in PSUM by a rank-1
     matmul, mm1 accumulates on top)            (PE)
  5. e = exp(logits), s = row-sum via ACT accum (ACT)
  6. a = e * (1/s)  (bf16)                      (DVE)
  7. ax[k, 0:128] += a.T @ [xn | ones] accumulated in PSUM over all 64
     tiles of the batch; column 128 gives a_sum (PE)
Finalize per batch: vlad = ax - centroids*a_sum, intra-L2-normalize rows,
scale by 1/sqrt(K)=0.125 for the global L2 norm (rows are unit after
intra-norm, so the global norm is sqrt(K) exactly up to fp rounding).
"""

import functools
from contextlib import ExitStack

import numpy as np

import concourse.bass as bass
import concourse.tile as tile
from concourse import bacc, masks, mybir
from concourse.bass_utils import run_bass_kernel_spmd

B, N, D, K = 32, 8192, 128, 64
NCORES = 8
BPC = B // NCORES            # 4 batches per core
P = 128                      # token tile size = partitions
NT_CHUNK = 16                # token tiles per DMA chunk (2048 tokens, 1 MiB)
NT_GROUP = 8                 # token tiles per softmax/psum group
TILES = N // P               # 64 token tiles per batch
CHUNKS = TILES // NT_CHUNK   # 4
GROUPS = NT_CHUNK // NT_GROUP  # 2 groups per chunk

F32 = mybir.dt.float32
BF16 = mybir.dt.bfloat16
EPS = 1e-12
MULT = mybir.AluOpType.mult
ADD = mybir.AluOpType.add


def _patch_act_tables():
    """Bias the ACT table-set chooser so Exp and Ln resolve to the one set
    that contains both ('natural_log_exp_and_others') - otherwise every
    Ln<->Exp alternation inserts a ~1.3-2.7us table reload. Order and size
    of the table list are preserved, so act_func_set_id stays valid."""
    import functools

    from concourse import bacc as _bacc, bass_interp as _bi, hw_specs as _hw

    if getattr(_hw, "_nv_patched", False):
        return
    orig = _hw.get_activation_tables

    @functools.cache
    def patched(arch):
        tabs = {k: set(v) for k, v in orig(arch).items()}
        both = "natural_log_exp_and_others"
        if both in tabs:
            drop = {
                mybir.ActivationFunctionType.Exp,
                mybir.ActivationFunctionType.Ln,
            }
            for name, fns in tabs.items():
                if name != both:
                    fns.difference_update(drop)
        return tabs

    _hw.get_activation_tables = patched
    _hw._nv_patched = True
    _bacc.get_activation_tables = patched
    _bi.get_activation_tables = patched


def _build_kernel(bpc=BPC, n=N, num_devices=NCORES, repeat=1):
    _patch_act_tables()
    tiles = n // P
    chunks = tiles // NT_CHUNK
    assert chunks * NT_CHUNK == tiles
    nc = bacc.Bacc(
        "TRN2", target_bir_lowering=False, debug=False, num_devices=num_devices
    )
    x_d = nc.dram_tensor("x", [bpc, n, D], F32, kind="ExternalInput").ap()
    cent_d = nc.dram_tensor("centroids", [K, D], F32, kind="ExternalInput").ap()
    cw_d = nc.dram_tensor("conv_w", [K, D], F32, kind="ExternalInput").ap()
    cb_d = nc.dram_tensor("conv_b", [1, K], F32, kind="ExternalInput").ap()
    y_d = nc.dram_tensor("y", [bpc, K, D], F32, kind="ExternalOutput").ap()

    with tile.TileContext(nc) as tc, ExitStack() as ctx:
        const = ctx.enter_context(tc.tile_pool(name="const", bufs=1))
        ident_bf = const.tile([P, P], BF16)
        masks.make_identity(nc, ident_bf[:])
        ident_f32 = const.tile([P, P], F32)
        masks.make_identity(nc, ident_f32[:])
        ones_row = const.tile([1, P], BF16)
        nc.gpsimd.memset(ones_row[:], 1.0)
        ln8 = const.tile([K, 1], F32)
        nc.gpsimd.memset(ln8[:], float(np.log(0.125)))

        cent_sb = const.tile([K, D], F32)
        nc.sync.dma_start(cent_sb[:], cent_d)
        cw_sb = const.tile([K, D], F32)
        nc.sync.dma_start(cw_sb[:], cw_d)
        cb_f32 = const.tile([1, K], F32)
        nc.sync.dma_start(cb_f32[:], cb_d)
        cb_bf = const.tile([1, K], BF16)
        nc.vector.tensor_copy(cb_bf[:], cb_f32[:])

        # wT = conv_w.T  [D, K] in bf16 (one-time PE transpose)
        ps_init = ctx.enter_context(
            tc.tile_pool(name="ps_init", bufs=1, space="PSUM")
        )
        cwT_ps = ps_init.tile([D, K], F32)
        nc.tensor.transpose(cwT_ps[:], cw_sb[:], ident_f32[0:K, 0:K])
        wT_bf = const.tile([D, K], BF16)
        nc.vector.tensor_copy(wT_bf[:], cwT_ps[:])

        x_pool = ctx.enter_context(tc.tile_pool(name="x", bufs=2))
        xn_pool = ctx.enter_context(tc.tile_pool(name="xn", bufs=2))
        sq_pool = ctx.enter_context(tc.tile_pool(name="sq", bufs=2))
        stat_pool = ctx.enter_context(tc.tile_pool(name="stat", bufs=3))
        e_pool = ctx.enter_context(tc.tile_pool(name="e", bufs=NT_GROUP + 2))
        ep_pool = ctx.enter_context(tc.tile_pool(name="ep", bufs=3))
        xts_pool = ctx.enter_context(tc.tile_pool(name="xts", bufs=3))
        fin_pool = ctx.enter_context(tc.tile_pool(name="fin", bufs=2))

        xt_psum = ctx.enter_context(tc.tile_pool(name="xt_ps", bufs=2, space="PSUM"))
        lg_psum = ctx.enter_context(tc.tile_pool(name="lg_ps", bufs=2, space="PSUM"))
        ax_psum = ctx.enter_context(tc.tile_pool(name="ax_ps", bufs=2, space="PSUM"))

        rep_ctx = tc.For_i(0, repeat, 1) if repeat > 1 else None
        if rep_ctx is not None:
            rep_ctx.__enter__()

        for b in range(bpc):
            ax_ps = ax_psum.tile([K, D + 1], F32, tag="ax")
            jj = 0
            for c in range(chunks):
                xs = x_pool.tile([P, NT_CHUNK, D], F32, tag="xs")
                src = x_d[b, c * NT_CHUNK * P:(c + 1) * NT_CHUNK * P, :]
                nc.sync.dma_start(xs[:], src.rearrange("(nt p) d -> p nt d", p=P))
                xn = xn_pool.tile([P, NT_CHUNK, D + 1], BF16, tag="xn")
                nc.gpsimd.memset(xn[:, :, D:D + 1], 1.0)

                for g in range(GROUPS):
                    ss = stat_pool.tile([P, NT_GROUP], F32, tag="ss")
                    for j in range(NT_GROUP):
                        t = g * NT_GROUP + j
                        sq = sq_pool.tile([P, D], F32, tag="sq")
                        nc.vector.scalar_tensor_tensor(
                            out=sq[:], in0=xs[:, t, :], scalar=1.0,
                            in1=xs[:, t, :], op0=MULT, op1=MULT,
                            accum_out=ss[:, j:j + 1],
                        )
                    # rnorm = 1/sqrt(ss) = exp(-0.5*ln(ss)): keeps every ACT
                    # op in the natural_log_exp table set (no table reloads)
                    nrm = stat_pool.tile([P, NT_GROUP], F32, tag="nrm")
                    nc.scalar.activation(
                        nrm[:], ss[:], mybir.ActivationFunctionType.Ln
                    )
                    rnorm = stat_pool.tile([P, NT_GROUP], F32, tag="rnorm")
                    nc.scalar.activation(
                        rnorm[:], nrm[:], mybir.ActivationFunctionType.Exp,
                        scale=-0.5,
                    )

                    lg = lg_psum.tile([P, NT_GROUP * K], F32, tag="lg")
                    s8 = stat_pool.tile([P, NT_GROUP], F32, tag="s8")
                    e_tiles = []
                    for j in range(NT_GROUP):
                        t = g * NT_GROUP + j
                        nc.gpsimd.tensor_scalar(
                            out=xn[:, t, 0:D], in0=xs[:, t, :],
                            scalar1=rnorm[:, j:j + 1], scalar2=None, op0=MULT,
                        )
                        xt_ps = xt_psum.tile([P, P], BF16, tag="xtp")
                        nc.tensor.transpose(xt_ps[:], xn[:, t, 0:D], ident_bf[:])
                        xt_sb = xts_pool.tile([P, P], BF16, tag="xts")
                        nc.vector.tensor_copy(xt_sb[:], xt_ps[:])
                        nc.tensor.matmul(
                            lg[:, j * K:(j + 1) * K], lhsT=ones_row[:],
                            rhs=cb_bf[:], start=True, stop=False,
                        )
                        nc.tensor.matmul(
                            lg[:, j * K:(j + 1) * K], lhsT=xt_sb[:],
                            rhs=wT_bf[:], start=False, stop=True,
                        )
                        e_sb = e_pool.tile([P, K], BF16, tag="e")
                        nc.scalar.activation(
                            e_sb[:], lg[:, j * K:(j + 1) * K],
                            mybir.ActivationFunctionType.Exp,
                            accum_out=s8[:, j:j + 1],
                        )
                        e_tiles.append(e_sb)
                    r8 = stat_pool.tile([P, NT_GROUP], F32, tag="r8")
                    nc.vector.reciprocal(r8[:], s8[:])
                    for j in range(NT_GROUP):
                        t = g * NT_GROUP + j
                        ep = ep_pool.tile([P, K], BF16, tag="ep")
                        nc.vector.tensor_scalar_mul(
                            ep[:], e_tiles[j][:], r8[:, j:j + 1]
                        )
                        nc.tensor.matmul(
                            ax_ps[:], lhsT=ep[:], rhs=xn[:, t, :],
                            start=(jj == 0), stop=(jj == tiles - 1),
                        )
                        jj += 1

            # ---- finalize batch b ----
            asum_sb = fin_pool.tile([K, 1], F32, tag="asum")
            nc.vector.tensor_copy(asum_sb[:], ax_ps[:, D:D + 1])
            tmp = fin_pool.tile([K, D], F32, tag="tmp")
            nc.vector.tensor_scalar_mul(tmp[:], cent_sb[:], asum_sb[:])
            vlad = fin_pool.tile([K, D], F32, tag="vlad")
            nc.vector.tensor_sub(vlad[:], ax_ps[:, 0:D], tmp[:])
            sqv = fin_pool.tile([K, D], F32, tag="sqv")
            rss = fin_pool.tile([K, 1], F32, tag="rss")
            nc.vector.scalar_tensor_tensor(
                out=sqv[:], in0=vlad[:], scalar=1.0, in1=vlad[:],
                op0=MULT, op1=MULT, accum_out=rss[:],
            )
            # 0.125/sqrt(rss) = exp(-0.5*ln(rss) + ln(0.125))
            nrm2 = fin_pool.tile([K, 1], F32, tag="nrm2")
            nc.scalar.activation(
                nrm2[:], rss[:], mybir.ActivationFunctionType.Ln
            )
            rn2 = fin_pool.tile([K, 1], F32, tag="rn2")
            nc.scalar.activation(
                rn2[:], nrm2[:], mybir.ActivationFunctionType.Exp,
                scale=-0.5, bias=ln8[:],
            )
            yb = fin_pool.tile([K, D], F32, tag="yb")
            nc.vector.tensor_scalar_mul(yb[:], vlad[:], rn2[:])
            nc.sync.dma_start(y_d[b], yb[:])

        if rep_ctx is not None:
            rep_ctx.__exit__(None, None, None)

    nc.compile()
    return nc


@functools.cache
def _get_kernel():
    return _build_kernel()


def kernel(x, centroids, conv_w, conv_b, **kw):
    x = np.ascontiguousarray(np.asarray(x, dtype=np.float32))
    centroids = np.ascontiguousarray(np.asarray(centroids, dtype=np.float32))
    conv_w = np.ascontiguousarray(np.asarray(conv_w, dtype=np.float32))
    conv_b = np.ascontiguousarray(
        np.asarray(conv_b, dtype=np.float32).reshape(1, K)
    )
    nc = _get_kernel()
    in_maps = [
        {
            "x": x[i * BPC:(i + 1) * BPC],
            "centroids": centroids,
            "conv_w": conv_w,
            "conv_b": conv_b,
        }
        for i in range(NCORES)
    ]
    res = run_bass_kernel_spmd(nc, in_maps, core_ids=list(range(NCORES)))
    y = np.concatenate([res.results[i]["y"] for i in range(NCORES)], axis=0)
    return y.reshape(B, K * D)


if __name__ == "__main__":
    rng = np.random.default_rng(0)
    out = kernel(
        x=rng.standard_normal((B, N, D), dtype=np.float32),
        centroids=rng.standard_normal((K, D), dtype=np.float32) * 0.01,
        conv_w=rng.standard_normal((K, D), dtype=np.float32) / np.sqrt(D),
        conv_b=rng.standard_normal((K,), dtype=np.float32) * 0.01,
    )
    print(out.shape, out.dtype, float(np.abs(out).max()))


# revision 14
# speedup vs baseline: 2.8523x; 2.6846x over previous
"""NetVLAD forward on 8 Trainium2 NeuronCores (Bass/Tile).

Data-parallel over batch: B=32 -> 4 batches per core. Per batch, stream
N=8192 tokens in 128-token tiles:
  1. sumsq + rsqrt -> per-token rnorm           (DVE/ACT)
  2. xn = x * rnorm  (bf16, fused f32->bf16)    (GPSIMD)
  3. xnT = PE-transpose(xn) -> PSUM -> SBUF     (PE + DVE copy)
  4. logits = xnT.T @ wT + conv_b  (bias pre-filled in PSUM by a rank-1
     matmul, mm1 accumulates on top)            (PE)
  5. e = exp(logits), s = row-sum via ACT accum (ACT)
  6. a = e * (1/s)  (bf16)                      (DVE)
  7. ax[k, 0:128] += a.T @ [xn | ones] accumulated in PSUM over all 64
     tiles of the batch; column 128 gives a_sum (PE)
Finalize per batch: vlad = ax - centroids*a_sum, intra-L2-normalize rows,
scale by 1/sqrt(K)=0.125 for the global L2 norm (rows are unit after
intra-norm, so the global norm is sqrt(K) exactly up to fp rounding).
"""

import functools
from contextlib import ExitStack

import numpy as np

import concourse.bass as bass
import concourse.tile as tile
from concourse import bacc, masks, mybir
from concourse.bass_utils import run_bass_kernel_spmd

B, N, D, K = 32, 8192, 128, 64
NCORES = 8
BPC = B // NCORES            # 4 batches per core
P = 128                      # token tile size = partitions
NT_CHUNK = 16                # token tiles per DMA chunk (2048 tokens, 1 MiB)
NT_GROUP = 8                 # token tiles per softmax/psum group
TILES = N // P               # 64 token tiles per batch
CHUNKS = TILES // NT_CHUNK   # 4
GROUPS = NT_CHUNK // NT_GROUP  # 2 groups per chunk

F32 = mybir.dt.float32
BF16 = mybir.dt.bfloat16
EPS = 1e-12
MULT = mybir.AluOpType.mult
ADD = mybir.AluOpType.add


def _patch_act_tables():
    """Bias the ACT table-set chooser so Exp and Ln resolve to the one set
    that contains both ('natural_log_exp_and_others') - otherwise every
    Ln<->Exp alternation inserts a ~1.3-2.7us table reload. Order and size
    of the table list are preserved, so act_func_set_id stays valid."""
    import functools

    from concourse import bacc as _bacc, bass_interp as _bi, hw_specs as _hw

    if getattr(_hw, "_nv_patched", False):
        return
    orig = _hw.get_activation_tables

    @functools.cache
    def patched(arch):
        tabs = {k: set(v) for k, v in orig(arch).items()}
        both = "natural_log_exp_and_others"
        if both in tabs:
            drop = {
                mybir.ActivationFunctionType.Exp,
                mybir.ActivationFunctionType.Ln,
            }
            for name, fns in tabs.items():
                if name != both:
                    fns.difference_update(drop)
        return tabs

    _hw.get_activation_tables = patched
    _hw._nv_patched = True
    _bacc.get_activation_tables = patched
    _bi.get_activation_tables = patched


def _build_kernel(bpc=BPC, n=N, num_devices=NCORES, repeat=1):
    _patch_act_tables()
    import os
    tiles = n // P
    chunks = tiles // NT_CHUNK
    assert chunks * NT_CHUNK == tiles
    nc = bacc.Bacc(
        "TRN2", target_bir_lowering=False, debug=False, num_devices=num_devices
    )
    x_d = nc.dram_tensor("x", [bpc, n, D], F32, kind="ExternalInput").ap()
    cent_d = nc.dram_tensor("centroids", [K, D], F32, kind="ExternalInput").ap()
    cw_d = nc.dram_tensor("conv_w", [K, D], F32, kind="ExternalInput").ap()
    cb_d = nc.dram_tensor("conv_b", [1, K], F32, kind="ExternalInput").ap()
    y_d = nc.dram_tensor("y", [bpc, K, D], F32, kind="ExternalOutput").ap()

    with tile.TileContext(nc) as tc, ExitStack() as ctx:
        const = ctx.enter_context(tc.tile_pool(name="const", bufs=1))
        ident_bf = const.tile([P, P], BF16)
        masks.make_identity(nc, ident_bf[:])
        ident_f32 = const.tile([P, P], F32)
        masks.make_identity(nc, ident_f32[:])
        ones_row = const.tile([1, P], BF16)
        nc.gpsimd.memset(ones_row[:], 1.0)
        ln8 = const.tile([K, 1], F32)
        nc.gpsimd.memset(ln8[:], float(np.log(0.125)))

        cent_sb = const.tile([K, D], F32)
        nc.sync.dma_start(cent_sb[:], cent_d)
        cw_sb = const.tile([K, D], F32)
        nc.sync.dma_start(cw_sb[:], cw_d)
        cb_f32 = const.tile([1, K], F32)
        nc.sync.dma_start(cb_f32[:], cb_d)
        cb_bf = const.tile([1, K], BF16)
        nc.vector.tensor_copy(cb_bf[:], cb_f32[:])

        # wT = conv_w.T  [D, K] in bf16 (one-time PE transpose)
        ps_init = ctx.enter_context(
            tc.tile_pool(name="ps_init", bufs=1, space="PSUM")
        )
        cwT_ps = ps_init.tile([D, K], F32)
        nc.tensor.transpose(cwT_ps[:], cw_sb[:], ident_f32[0:K, 0:K])
        wT_bf = const.tile([D, K], BF16)
        nc.vector.tensor_copy(wT_bf[:], cwT_ps[:])

        x_pool = ctx.enter_context(tc.tile_pool(name="x", bufs=2))
        xn_pool = ctx.enter_context(tc.tile_pool(name="xn", bufs=2))
        sq_pool = ctx.enter_context(tc.tile_pool(name="sq", bufs=2))
        stat_pool = ctx.enter_context(tc.tile_pool(name="stat", bufs=3))
        e_pool = ctx.enter_context(tc.tile_pool(name="e", bufs=NT_GROUP + 2))
        ep_pool = ctx.enter_context(tc.tile_pool(name="ep", bufs=3))
        xts_pool = ctx.enter_context(tc.tile_pool(name="xts", bufs=3))
        fin_pool = ctx.enter_context(tc.tile_pool(name="fin", bufs=2))

        xt_psum = ctx.enter_context(tc.tile_pool(name="xt_ps", bufs=2, space="PSUM"))
        lg_psum = ctx.enter_context(tc.tile_pool(name="lg_ps", bufs=2, space="PSUM"))
        ax_psum = ctx.enter_context(tc.tile_pool(name="ax_ps", bufs=2, space="PSUM"))

        rep_ctx = tc.For_i(0, repeat, 1) if repeat > 1 else None
        if rep_ctx is not None:
            rep_ctx.__enter__()

        for b in range(bpc):
            ax_ps = ax_psum.tile([K, D + 1], F32, tag="ax")
            jj = 0
            for c in range(chunks):
                xs = x_pool.tile([P, NT_CHUNK, D], F32, tag="xs")
                src = x_d[b, c * NT_CHUNK * P:(c + 1) * NT_CHUNK * P, :]
                nc.sync.dma_start(xs[:], src.rearrange("(nt p) d -> p nt d", p=P))
                xn = xn_pool.tile([P, NT_CHUNK, D + 1], BF16, tag="xn")
                nc.gpsimd.memset(xn[:, :, D:D + 1], 1.0)

                XN_ENGINE = (nc.vector.tensor_scalar if os.environ.get("XN_DVE")
                             else nc.gpsimd.tensor_scalar)
                for g in range(GROUPS):
                    ss = stat_pool.tile([P, NT_GROUP], F32, tag="ss")
                    for j in range(NT_GROUP):
                        t = g * NT_GROUP + j
                        sq = sq_pool.tile([P, D], F32, tag="sq")
                        nc.vector.scalar_tensor_tensor(
                            out=sq[:], in0=xs[:, t, :], scalar=1.0,
                            in1=xs[:, t, :], op0=MULT, op1=MULT,
                            accum_out=ss[:, j:j + 1],
                        )
                    # rnorm = 1/sqrt(ss) = exp(-0.5*ln(ss)): keeps every ACT
                    # op in the natural_log_exp table set (no table reloads)
                    nrm = stat_pool.tile([P, NT_GROUP], F32, tag="nrm")
                    nc.scalar.activation(
                        nrm[:], ss[:], mybir.ActivationFunctionType.Ln
                    )
                    rnorm = stat_pool.tile([P, NT_GROUP], F32, tag="rnorm")
                    nc.scalar.activation(
                        rnorm[:], nrm[:], mybir.ActivationFunctionType.Exp,
                        scale=-0.5,
                    )

                    lg = lg_psum.tile([P, NT_GROUP * K], F32, tag="lg")
                    s8 = stat_pool.tile([P, NT_GROUP], F32, tag="s8")
                    e_tiles = []
                    for j in range(NT_GROUP):
                        t = g * NT_GROUP + j
                        XN_ENGINE(
                            out=xn[:, t, 0:D], in0=xs[:, t, :],
                            scalar1=rnorm[:, j:j + 1], scalar2=None, op0=MULT,
                        )
                        xt_ps = xt_psum.tile([P, P], BF16, tag="xtp")
                        nc.tensor.transpose(xt_ps[:], xn[:, t, 0:D], ident_bf[:])
                        xt_sb = xts_pool.tile([P, P], BF16, tag="xts")
                        nc.vector.tensor_copy(xt_sb[:], xt_ps[:])
                        nc.tensor.matmul(
                            lg[:, j * K:(j + 1) * K], lhsT=ones_row[:],
                            rhs=cb_bf[:], start=True, stop=False,
                        )
                        nc.tensor.matmul(
                            lg[:, j * K:(j + 1) * K], lhsT=xt_sb[:],
                            rhs=wT_bf[:], start=False, stop=True,
                        )
                        e_sb = e_pool.tile([P, K], BF16, tag="e")
                        nc.scalar.activation(
                            e_sb[:], lg[:, j * K:(j + 1) * K],
                            mybir.ActivationFunctionType.Exp,
                            accum_out=s8[:, j:j + 1],
                        )
                        e_tiles.append(e_sb)
                    r8 = stat_pool.tile([P, NT_GROUP], F32, tag="r8")
                    nc.vector.reciprocal(r8[:], s8[:])
                    for j in range(NT_GROUP):
                        t = g * NT_GROUP + j
                        ep = ep_pool.tile([P, K], BF16, tag="ep")
                        nc.vector.tensor_scalar_mul(
                            ep[:], e_tiles[j][:], r8[:, j:j + 1]
                        )
                        nc.tensor.matmul(
                            ax_ps[:], lhsT=ep[:], rhs=xn[:, t, :],
                            start=(jj == 0), stop=(jj == tiles - 1),
                        )
                        jj += 1

            # ---- finalize batch b ----
            asum_sb = fin_pool.tile([K, 1], F32, tag="asum")
            nc.vector.tensor_copy(asum_sb[:], ax_ps[:, D:D + 1])
            tmp = fin_pool.tile([K, D], F32, tag="tmp")
            nc.vector.tensor_scalar_mul(tmp[:], cent_sb[:], asum_sb[:])
            vlad = fin_pool.tile([K, D], F32, tag="vlad")
            nc.vector.tensor_sub(vlad[:], ax_ps[:, 0:D], tmp[:])
            sqv = fin_pool.tile([K, D], F32, tag="sqv")
            rss = fin_pool.tile([K, 1], F32, tag="rss")
            nc.vector.scalar_tensor_tensor(
                out=sqv[:], in0=vlad[:], scalar=1.0, in1=vlad[:],
                op0=MULT, op1=MULT, accum_out=rss[:],
            )
            # 0.125/sqrt(rss) = exp(-0.5*ln(rss) + ln(0.125))
            nrm2 = fin_pool.tile([K, 1], F32, tag="nrm2")
            nc.scalar.activation(
                nrm2[:], rss[:], mybir.ActivationFunctionType.Ln
            )
            rn2 = fin_pool.tile([K, 1], F32, tag="rn2")
            nc.scalar.activation(
                rn2[:], nrm2[:], mybir.ActivationFunctionType.Exp,
                scale=-0.5, bias=ln8[:],
            )
            yb = fin_pool.tile([K, D], F32, tag="yb")
            nc.vector.tensor_scalar_mul(yb[:], vlad[:], rn2[:])
            nc.sync.dma_start(y_d[b], yb[:])

        if rep_ctx is not None:
            rep_ctx.__exit__(None, None, None)

    nc.compile()
    return nc


@functools.cache
def _get_kernel():
    return _build_kernel()


def kernel(x, centroids, conv_w, conv_b, **kw):
    x = np.ascontiguousarray(np.asarray(x, dtype=np.float32))
    centroids = np.ascontiguousarray(np.asarray(centroids, dtype=np.float32))
    conv_w = np.ascontiguousarray(np.asarray(conv_w, dtype=np.float32))
    conv_b = np.ascontiguousarray(
        np.asarray(conv_b, dtype=np.float32).reshape(1, K)
    )
    nc = _get_kernel()
    in_maps = [
        {
            "x": x[i * BPC:(i + 1) * BPC],
            "centroids": centroids,
            "conv_w": conv_w,
            "conv_b": conv_b,
        }
        for i in range(NCORES)
    ]
    res = run_bass_kernel_spmd(nc, in_maps, core_ids=list(range(NCORES)))
    y = np.concatenate([res.results[i]["y"] for i in range(NCORES)], axis=0)
    return y.reshape(B, K * D)


if __name__ == "__main__":
    rng = np.random.default_rng(0)
    out = kernel(
        x=rng.standard_normal((B, N, D), dtype=np.float32),
        centroids=rng.standard_normal((K, D), dtype=np.float32) * 0.01,
        conv_w=rng.standard_normal((K, D), dtype=np.float32) / np.sqrt(D),
        conv_b=rng.standard_normal((K,), dtype=np.float32) * 0.01,
    )
    print(out.shape, out.dtype, float(np.abs(out).max()))


# revision 18
# speedup vs baseline: 3.1802x; 1.1149x over previous
"""NetVLAD forward on 8 Trainium2 NeuronCores (Bass/Tile).

Data-parallel over batch: B=32 -> 4 batches per core. Per batch, stream
N=8192 tokens in 128-token tiles:
  1. sumsq + rsqrt -> per-token rnorm           (DVE/ACT)
  2. xn = x * rnorm  (bf16, fused f32->bf16)    (GPSIMD)
  3. xnT = PE-transpose(xn) -> PSUM -> SBUF     (PE + DVE copy)
  4. logits = xnT.T @ wT + conv_b  (bias pre-filled in PSUM by a rank-1
     matmul, mm1 accumulates on top)            (PE)
  5. e = exp(logits), s = row-sum via ACT accum (ACT)
  6. a = e * (1/s)  (bf16)                      (DVE)
  7. ax[k, 0:128] += a.T @ [xn | ones] accumulated in PSUM over all 64
     tiles of the batch; column 128 gives a_sum (PE)
Finalize per batch: vlad = ax - centroids*a_sum, intra-L2-normalize rows,
scale by 1/sqrt(K)=0.125 for the global L2 norm (rows are unit after
intra-norm, so the global norm is sqrt(K) exactly up to fp rounding).
"""

import functools
from contextlib import ExitStack

import numpy as np

import concourse.bass as bass
import concourse.tile as tile
from concourse import bacc, masks, mybir
from concourse.bass_utils import run_bass_kernel_spmd

B, N, D, K = 32, 8192, 128, 64
NCORES = 8
BPC = B // NCORES            # 4 batches per core
P = 128                      # token tile size = partitions
NT_CHUNK = 16                # token tiles per DMA chunk (2048 tokens, 1 MiB)
NT_GROUP = 8                 # token tiles per softmax/psum group
TILES = N // P               # 64 token tiles per batch
CHUNKS = TILES // NT_CHUNK   # 4
GROUPS = NT_CHUNK // NT_GROUP  # 2 groups per chunk

F32 = mybir.dt.float32
BF16 = mybir.dt.bfloat16
EPS = 1e-12
MULT = mybir.AluOpType.mult
ADD = mybir.AluOpType.add


def _patch_act_tables():
    """Bias the ACT table-set chooser so Exp and Ln resolve to the one set
    that contains both ('natural_log_exp_and_others') - otherwise every
    Ln<->Exp alternation inserts a ~1.3-2.7us table reload. Order and size
    of the table list are preserved, so act_func_set_id stays valid."""
    import functools

    from concourse import bacc as _bacc, bass_interp as _bi, hw_specs as _hw

    if getattr(_hw, "_nv_patched", False):
        return
    orig = _hw.get_activation_tables

    @functools.cache
    def patched(arch):
        tabs = {k: set(v) for k, v in orig(arch).items()}
        both = "natural_log_exp_and_others"
        if both in tabs:
            drop = {
                mybir.ActivationFunctionType.Exp,
                mybir.ActivationFunctionType.Ln,
            }
            for name, fns in tabs.items():
                if name != both:
                    fns.difference_update(drop)
        return tabs

    _hw.get_activation_tables = patched
    _hw._nv_patched = True
    _bacc.get_activation_tables = patched
    _bi.get_activation_tables = patched


def _build_kernel(bpc=BPC, n=N, num_devices=NCORES, repeat=1):
    _patch_act_tables()
    tiles = n // P
    chunks = tiles // NT_CHUNK
    assert chunks * NT_CHUNK == tiles
    nc = bacc.Bacc(
        "TRN2", target_bir_lowering=False, debug=False, num_devices=num_devices
    )
    x_d = nc.dram_tensor("x", [bpc, n, D], F32, kind="ExternalInput").ap()
    cent_d = nc.dram_tensor("centroids", [K, D], F32, kind="ExternalInput").ap()
    cw_d = nc.dram_tensor("conv_w", [K, D], F32, kind="ExternalInput").ap()
    cb_d = nc.dram_tensor("conv_b", [1, K], F32, kind="ExternalInput").ap()
    y_d = nc.dram_tensor("y", [bpc, K, D], F32, kind="ExternalOutput").ap()

    with tile.TileContext(nc) as tc, ExitStack() as ctx:
        const = ctx.enter_context(tc.tile_pool(name="const", bufs=1))
        ident_bf = const.tile([P, P], BF16)
        masks.make_identity(nc, ident_bf[:])
        ident_f32 = const.tile([P, P], F32)
        masks.make_identity(nc, ident_f32[:])
        ones_row = const.tile([1, P], BF16)
        nc.gpsimd.memset(ones_row[:], 1.0)
        ln8 = const.tile([K, 1], F32)
        nc.gpsimd.memset(ln8[:], float(np.log(0.125)))

        cent_sb = const.tile([K, D], F32)
        nc.sync.dma_start(cent_sb[:], cent_d)
        cw_sb = const.tile([K, D], F32)
        nc.sync.dma_start(cw_sb[:], cw_d)
        cb_f32 = const.tile([1, K], F32)
        nc.sync.dma_start(cb_f32[:], cb_d)
        cb8 = const.tile([1, NT_GROUP * K], BF16)
        for _j in range(NT_GROUP):
            nc.vector.tensor_copy(cb8[:, _j * K:(_j + 1) * K], cb_f32[:])

        # wT = conv_w.T  [D, K] in bf16 (one-time PE transpose)
        ps_init = ctx.enter_context(
            tc.tile_pool(name="ps_init", bufs=1, space="PSUM")
        )
        cwT_ps = ps_init.tile([D, K], F32)
        nc.tensor.transpose(cwT_ps[:], cw_sb[:], ident_f32[0:K, 0:K])
        wT_bf = const.tile([D, K], BF16)
        nc.vector.tensor_copy(wT_bf[:], cwT_ps[:])

        x_pool = ctx.enter_context(tc.tile_pool(name="x", bufs=2))
        xn_pool = ctx.enter_context(tc.tile_pool(name="xn", bufs=2))
        sq_pool = ctx.enter_context(tc.tile_pool(name="sq", bufs=2))
        stat_pool = ctx.enter_context(tc.tile_pool(name="stat", bufs=3))
        e_pool = ctx.enter_context(tc.tile_pool(name="e", bufs=2))
        ep_pool = ctx.enter_context(tc.tile_pool(name="ep", bufs=2))
        xts_pool = ctx.enter_context(tc.tile_pool(name="xts", bufs=2))
        fin_pool = ctx.enter_context(tc.tile_pool(name="fin", bufs=2))

        xt_psum = ctx.enter_context(tc.tile_pool(name="xt_ps", bufs=2, space="PSUM"))
        lg_psum = ctx.enter_context(tc.tile_pool(name="lg_ps", bufs=2, space="PSUM"))
        ax_psum = ctx.enter_context(tc.tile_pool(name="ax_ps", bufs=2, space="PSUM"))

        rep_ctx = tc.For_i(0, repeat, 1) if repeat > 1 else None
        if rep_ctx is not None:
            rep_ctx.__enter__()

        for b in range(bpc):
            ax_ps = ax_psum.tile([K, D + 1], F32, tag="ax")
            jj = 0
            for c in range(chunks):
                xs = x_pool.tile([P, NT_CHUNK, D], F32, tag="xs")
                src = x_d[b, c * NT_CHUNK * P:(c + 1) * NT_CHUNK * P, :]
                nc.sync.dma_start(xs[:], src.rearrange("(nt p) d -> p nt d", p=P))
                xn = xn_pool.tile([P, NT_CHUNK, D + 1], BF16, tag="xn")
                nc.vector.memset(xn[:, :, D:D + 1], 1.0)

                for g in range(GROUPS):
                    g0 = g * NT_GROUP
                    ss = stat_pool.tile([P, NT_GROUP], F32, tag="ss")
                    for j in range(NT_GROUP):
                        t = g0 + j
                        sq = sq_pool.tile([P, D], F32, tag="sq")
                        nc.vector.scalar_tensor_tensor(
                            out=sq[:], in0=xs[:, t, :], scalar=1.0,
                            in1=xs[:, t, :], op0=MULT, op1=MULT,
                            accum_out=ss[:, j:j + 1],
                        )
                    # rnorm = 1/sqrt(ss) = exp(-0.5*ln(ss)): keeps every ACT
                    # op in the natural_log_exp table set (no table reloads)
                    nrm = stat_pool.tile([P, NT_GROUP], F32, tag="nrm")
                    nc.scalar.activation(
                        nrm[:], ss[:], mybir.ActivationFunctionType.Ln
                    )
                    rnorm = stat_pool.tile([P, NT_GROUP], F32, tag="rnorm")
                    nc.scalar.activation(
                        rnorm[:], nrm[:], mybir.ActivationFunctionType.Exp,
                        scale=-0.5,
                    )

                    # xn = x * rnorm (bf16) for the whole group in one op
                    nc.vector.scalar_tensor_tensor(
                        out=xn[:, g0:g0 + NT_GROUP, 0:D],
                        in0=xs[:, g0:g0 + NT_GROUP, :], scalar=1.0,
                        in1=rnorm[:].broadcast_to([P, NT_GROUP, D]),
                        op0=MULT, op1=MULT,
                    )

                    lg = lg_psum.tile([P, NT_GROUP * K], F32, tag="lg")
                    s8 = stat_pool.tile([P, NT_GROUP], F32, tag="s8")
                    e_g = e_pool.tile([P, NT_GROUP * K], BF16, tag="e")
                    for h in range(NT_GROUP // 4):
                        xt_ps = xt_psum.tile([P, 4 * P], BF16, tag="xtp")
                        for q in range(4):
                            t = g0 + h * 4 + q
                            nc.tensor.transpose(
                                xt_ps[:, q * P:(q + 1) * P], xn[:, t, 0:D],
                                ident_bf[:],
                            )
                        xt_sb = xts_pool.tile([P, 4 * P], BF16, tag="xts")
                        nc.vector.tensor_copy(xt_sb[:], xt_ps[:])
                        for q in range(4):
                            j = h * 4 + q
                            nc.tensor.matmul(
                                lg[:, j * K:(j + 1) * K], lhsT=ones_row[:],
                                rhs=cb8[:, j * K:(j + 1) * K],
                                start=True, stop=False,
                            )
                            nc.tensor.matmul(
                                lg[:, j * K:(j + 1) * K],
                                lhsT=xt_sb[:, q * P:(q + 1) * P],
                                rhs=wT_bf[:], start=False, stop=True,
                            )
                            nc.scalar.activation(
                                e_g[:, j * K:(j + 1) * K],
                                lg[:, j * K:(j + 1) * K],
                                mybir.ActivationFunctionType.Exp,
                                accum_out=s8[:, j:j + 1],
                            )
                    r8 = stat_pool.tile([P, NT_GROUP], F32, tag="r8")
                    nc.vector.reciprocal(r8[:], s8[:])
                    # a = e * (1/s) for the whole group in one op
                    ep_g = ep_pool.tile([P, NT_GROUP * K], BF16, tag="ep")
                    nc.vector.scalar_tensor_tensor(
                        out=ep_g[:].rearrange("p (g k) -> p g k", g=NT_GROUP),
                        in0=e_g[:].rearrange("p (g k) -> p g k", g=NT_GROUP),
                        scalar=1.0,
                        in1=r8[:].broadcast_to([P, NT_GROUP, K]),
                        op0=MULT, op1=MULT,
                    )
                    for j in range(NT_GROUP):
                        t = g0 + j
                        nc.tensor.matmul(
                            ax_ps[:], lhsT=ep_g[:, j * K:(j + 1) * K],
                            rhs=xn[:, t, :],
                            start=(jj == 0), stop=(jj == tiles - 1),
                        )
                        jj += 1

            # ---- finalize batch b ----
            asum_sb = fin_pool.tile([K, 1], F32, tag="asum")
            nc.vector.tensor_copy(asum_sb[:], ax_ps[:, D:D + 1])
            tmp = fin_pool.tile([K, D], F32, tag="tmp")
            nc.vector.tensor_scalar_mul(tmp[:], cent_sb[:], asum_sb[:])
            vlad = fin_pool.tile([K, D], F32, tag="vlad")
            nc.vector.tensor_sub(vlad[:], ax_ps[:, 0:D], tmp[:])
            sqv = fin_pool.tile([K, D], F32, tag="sqv")
            rss = fin_pool.tile([K, 1], F32, tag="rss")
            nc.vector.scalar_tensor_tensor(
                out=sqv[:], in0=vlad[:], scalar=1.0, in1=vlad[:],
                op0=MULT, op1=MULT, accum_out=rss[:],
            )
            # 0.125/sqrt(rss) = exp(-0.5*ln(rss) + ln(0.125))
            nrm2 = fin_pool.tile([K, 1], F32, tag="nrm2")
            nc.scalar.activation(
                nrm2[:], rss[:], mybir.ActivationFunctionType.Ln
            )
            rn2 = fin_pool.tile([K, 1], F32, tag="rn2")
            nc.scalar.activation(
                rn2[:], nrm2[:], mybir.ActivationFunctionType.Exp,
                scale=-0.5, bias=ln8[:],
            )
            yb = fin_pool.tile([K, D], F32, tag="yb")
            nc.vector.tensor_scalar_mul(yb[:], vlad[:], rn2[:])
            nc.sync.dma_start(y_d[b], yb[:])

        if rep_ctx is not None:
            rep_ctx.__exit__(None, None, None)

    nc.compile()
    return nc


@functools.cache
def _get_kernel():
    return _build_kernel()


def kernel(x, centroids, conv_w, conv_b, **kw):
    x = np.ascontiguousarray(np.asarray(x, dtype=np.float32))
    centroids = np.ascontiguousarray(np.asarray(centroids, dtype=np.float32))
    conv_w = np.ascontiguousarray(np.asarray(conv_w, dtype=np.float32))
    conv_b = np.ascontiguousarray(
        np.asarray(conv_b, dtype=np.float32).reshape(1, K)
    )
    nc = _get_kernel()
    in_maps = [
        {
            "x": x[i * BPC:(i + 1) * BPC],
            "centroids": centroids,
            "conv_w": conv_w,
            "conv_b": conv_b,
        }
        for i in range(NCORES)
    ]
    res = run_bass_kernel_spmd(nc, in_maps, core_ids=list(range(NCORES)))
    y = np.concatenate([res.results[i]["y"] for i in range(NCORES)], axis=0)
    return y.reshape(B, K * D)


if __name__ == "__main__":
    rng = np.random.default_rng(0)
    out = kernel(
        x=rng.standard_normal((B, N, D), dtype=np.float32),
        centroids=rng.standard_normal((K, D), dtype=np.float32) * 0.01,
        conv_w=rng.standard_normal((K, D), dtype=np.float32) / np.sqrt(D),
        conv_b=rng.standard_normal((K,), dtype=np.float32) * 0.01,
    )
    print(out.shape, out.dtype, float(np.abs(out).max()))
